# revision 39
# baseline (speedup 1.0000x reference)
"""Causal self-attention Trainium2 kernel (v14, ~138us; v2 baseline was 175us).

Problem: B=2, T=2048, C=768, 12 heads of dim 64, fp32.
  qkv = x @ W_attn.T ; per-head causal softmax(Q K^T / 8) @ V ; y = attn @ W_proj.T

Sharding over 8 cores: core = b * 4 + g where b = batch (2), g = head-group
(4 groups x 3 heads).  Each core computes QKV for its 3 heads, causal
attention, and a partial projection y_partial[b] = attn[:, S_g] @ W_proj[:, S_g].T.
Host upcasts the bf16 partials and sums the 4 per batch.

What moved the needle from 175us (trace-driven, in order):
  - input DMA: host prepacks x into 4 per-tchunk [128, 6, 512] tensors and
    the weights into two [128, *] packs, all contiguous-per-partition (the
    old layouts produced 384B strided descriptors -> sub-50GB/s; inputs
    finished landing at ~30us, stalling the PE ~24us AND keeping the HAM
    clock gate cold until 35us).  Need-ordered queues: sync + gpsimd each
    carry half of every x tchunk (halves time-to-tchunk), scalar carries
    the weight packs on its own HWDGE ring.
  - PE warm-up: ~6us of junk matmuls at t=0 (during the DMA fill) trips
    the HAM SHORT activity window so real work starts at 2.4GHz, and a
    dummy exp pulls the ~2.7us ACT table load off the critical path.
  - ScalarE diet: head 2's exps are paired across two k-tiles (one
    [128,2,QC] activation per pair) -- 60 exp instructions instead of 80;
    the (N+352cyc)/1.2GHz per-instruction overhead made ScalarE the
    attention-phase bottleneck (72us busy -> 64us).  V-tile evacuations
    moved to DVE so ScalarE runs (almost) only exps.
  - head 2 score packing: qk2 is duplicated onto partitions 64:128 by a
    cheap SBUF->SBUF DMA (qk2u); each k-tile pair's two score matmuls then
    run CONCURRENTLY in array row groups 0:2 / 2:4 (like heads 0/1, whose
    [128, T] packing row-tiles automatically).  -9.4us.
  - att(0) has no previous chunk to project, so the x_t1 DMA wait is a
    genuine PE hole: a 15-matmul junk bridge is pinned BOTH behind att(0)
    blk0's last PV and AHEAD of vqk(1)'s first matmul (without the second
    pin the scheduler parks the DMA-blocked vqk matmuls in front and the
    bridge never covers the hole).  Keeps the clock gate warm from 12us
    to 90us straight; throttle_active 43us -> 23us.
  - chunk-boundary stalls: the next chunk's V/QK chains are emitted
    BETWEEN attention blk0 and blk1 so their DVE evacuations retire during
    blk1 and att(j+1)'s first scores never wait on the DVE FIFO; the 3.3us
    InstReciprocal is emitted after them for the same reason (DVE FIFO
    head-of-line blocking caused ~3us PE stalls per boundary, and each
    stall re-throttled the PE clock gate for multiple 3.4us windows).
  - tail: y partials in bf16 (halves the output drain); the last chunk's
    den copies and ot evacuations are DEFERRED behind the previous chunk's
    normalize/projection DVE work (they block at the DVE FIFO head until
    the last PV anyway); a single 24-matmul junk bridge pinned behind the
    last PV keeps the PE clock warm across the reciprocal wait; then
    reciprocal/normalize/projection/drain run pipelined per 128-column
    quarter, evacuations alternating ScalarE/DVE, the final tile split
    across both HWDGE rings.

Layout (no on-device transposes anywhere):
  - host passes x[b].T in tchunk packs, W slices pre-transposed, all bf16.
  - Q^T, K^T d-major; heads 0,1 packed [128, T]; head 2's q/k share one
    [64, 2, T] tile + the qk2u duplicate.  V t-major with an appended
    ones-column so the P@V matmul also emits the softmax denominator as
    its output row 64 (this forces M=65 and keeps PV serial -- every
    cheaper denominator alternative loses more than it gains).
  - scores are computed transposed, ST[k, q] = K Q^T; exp runs on ScalarE
    straight out of PSUM (no max-subtraction: |scores/8| < ~3, safe in
    fp32; masked lanes get -1e30 and underflow to exact 0).
  - causal masking ON TensorE: the diagonal tile's score group accumulates
    ident.T @ [mneg|mneg] for both blk0 heads in one strided matmul.
  - normalization: reciprocal of the denominator rows (exact InstReciprocal;
    the custom-DVE approx ops fail this walrus build's codegen), broadcast
    across partitions with a K<=65 outer-product matmul, one multiply per
    block; the tail quarters feed the fp32 reciprocal straight into fp32
    selector matmuls (no bf16 round-trip).
  - PSUM budget (the hard wall): st 2x[128,2,QC] double-buffered (4) +
    2 ot accumulators (2) + shared V/QK/proj/norm pool (2) = 8 banks.
    Every larger exp-merge or PV-packing scheme dies here.

Walrus rejects any engine instruction carrying >= 2 semaphore waits;
_split_excess_waits moves excess waits onto same-engine EventSemaphore
instructions.
"""

from contextlib import ExitStack

import numpy as np

import concourse.bass as bass
import concourse.mybir as mybir
from concourse.tile import TileContext
from concourse.tile_rust import add_dep_helper
from concourse.bass_utils import run_bass_kernel_spmd

B, T, C = 2, 2048, 768
NH = 12
HEAD = 64
HPC = 3              # heads per core
CP = HPC * HEAD      # 192 channels per core
SCALE = 1.0 / 8.0    # 1/sqrt(64)
NEG = -1.0e30

P = 128
TT = T // P          # 16 t-tiles
CK = C // P          # 6 contraction chunks over C
QC = 512             # q-chunk (one PSUM bank of fp32)
NQC = T // QC        # 4
F32 = mybir.dt.float32
BF16 = mybir.dt.bfloat16

# wlate column map
WP01 = 0             # [128, 768]
WP2 = 768            # [64, 768] at partitions 0:64
IDC = 1536           # ident [128, 128]
MNC = 1664           # mneg [128, 128]; mneg2 = [:, MNC:MNC+256]
SELC = 1920          # sel01 [33, 128]
WLW = 2048

N_WARM = 30          # junk matmuls at t=0 to trip the HAM SHORT window

_CACHED = {}


def _split_excess_waits(nc):
    """This walrus accepts at most 1 semaphore wait per instruction (2 on
    EventSemaphore).  Move excess waits onto same-engine EventSemaphore
    instructions inserted immediately before the overloaded instruction —
    sequencer FIFO order makes that semantically identical."""
    n = 0
    for f in nc.m.functions:
        for bb in f.blocks:
            out = []
            for inst in bb.instructions:
                tname = type(inst).__name__
                is_isa = tname == "InstISA"
                cap = 0 if is_isa else (2 if tname == "InstEventSemaphore" else 1)
                si = inst.sync_info
                if si is None:
                    out.append(inst)
                    continue
                waits = list(si.on_wait)
                upds = list(si.on_update)
                if len(waits) > cap or (is_isa and upds):
                    extra = waits[: len(waits) - cap] if len(waits) > cap else []
                    keep = waits[len(extra) :]
                    while extra:
                        chunk, extra = extra[:2], extra[2:]
                        n += 1
                        ev = mybir.InstEventSemaphore(
                            name=f"WSPLIT-{n}", engine=inst.engine
                        )
                        ev.sync_info = mybir.SyncInfo(on_wait=chunk, on_update=[])
                        out.append(ev)
                    post = []
                    if is_isa and upds:
                        n += 1
                        ev = mybir.InstEventSemaphore(
                            name=f"WSPLIT-{n}", engine=inst.engine
                        )
                        ev.sync_info = mybir.SyncInfo(on_wait=[], on_update=upds)
                        post.append(ev)
                        upds = []
                    inst.sync_info = mybir.SyncInfo(on_wait=keep, on_update=upds)
                    out.append(inst)
                    out.extend(post)
                else:
                    out.append(inst)
            bb.instructions = out
    return n


def _build():
    nc = bass.Bass()

    x_t = [
        nc.dram_tensor(f"x_t{t}", [P, CK, QC], BF16, kind="ExternalInput")
        for t in range(NQC)
    ]
    wqkv = nc.dram_tensor("wqkv", [P, CK, 576], BF16, kind="ExternalInput")
    wlate = nc.dram_tensor("wlate", [P, WLW], BF16, kind="ExternalInput")
    # bf16 partials: halves the 6.3MB output drain; the host accumulates the
    # four partials in fp32.  Adds ~2^-9-relative rounding per partial --
    # well inside the error budget (matmul operands are already bf16).
    y = nc.dram_tensor("y", [T, C], BF16, kind="ExternalOutput")

    Exp = mybir.ActivationFunctionType.Exp
    Copy = mybir.ActivationFunctionType.Copy

    with TileContext(nc) as tc, ExitStack() as stk:
        wpool = stk.enter_context(tc.tile_pool(name="weights", bufs=1))
        xpool = stk.enter_context(tc.tile_pool(name="xpool", bufs=1))
        vpool = stk.enter_context(tc.tile_pool(name="vpool", bufs=1))
        qkpool = stk.enter_context(tc.tile_pool(name="qkpool", bufs=1))
        otpool = stk.enter_context(tc.tile_pool(name="otpool", bufs=1))
        ptpool = stk.enter_context(tc.tile_pool(name="ptpool", bufs=3))
        misc = stk.enter_context(tc.tile_pool(name="misc", bufs=1))
        ypool = stk.enter_context(tc.tile_pool(name="ypool", bufs=4))
        ps_st = stk.enter_context(tc.tile_pool(name="ps_st", bufs=2, space="PSUM"))
        ps_ot = stk.enter_context(tc.tile_pool(name="ps_ot", bufs=1, space="PSUM"))
        ps_sm = stk.enter_context(tc.tile_pool(name="ps_sm", bufs=2, space="PSUM"))

        # ---- tiles ----
        w_sb = wpool.tile([P, CK, 576], BF16)
        wl_sb = wpool.tile([P, WLW], BF16)
        junk = wpool.tile([P, QC], BF16)
        ones_sb = wpool.tile([1, HEAD], BF16)
        x_sb = xpool.tile([P, CK, T], BF16, name="x_sb", tag="x_sb")
        v_sb = vpool.tile([P, TT, HPC, HEAD + 1], BF16)
        qt01 = qkpool.tile([P, T], BF16, name="qt01", tag="qt01")
        kt01 = qkpool.tile([P, T], BF16, name="kt01", tag="kt01")
        qk2 = qkpool.tile([HEAD, 2, T], BF16, name="qk2", tag="qk2")
        # duplicate of qk2 on partitions 64:128 (filled by SBUF->SBUF DMA):
        # lets the pair's second score matmul run in array row groups 2:4,
        # CONCURRENT with the first (row groups 0:2) -- halves blk1's score
        # span for zero PE/DVE cost.
        qk2u = qkpool.tile([P, 2, T], BF16, name="qk2u", tag="qk2u")
        ot01 = otpool.tile([P, T], BF16, name="ot01", tag="ot01")
        ot2 = otpool.tile([HEAD, T], BF16, name="ot2", tag="ot2")
        dummy = misc.tile([1, 8], F32, tag="dummy")
        den_sb = misc.tile([HEAD + 1, QC], F32, tag="den")
        sel32 = misc.tile([33, P], F32, tag="sel32")
        # row 64 -> ones: picks the h2 reciprocal row with a base-0 lhsT
        sel32b = misc.tile([HEAD + 1, HEAD], F32, tag="sel32b")

        # weight slices (views into the packs)
        def wq(ck):
            return w_sb[:, ck, 0:P]

        def wk(ck):
            return w_sb[:, ck, P : 2 * P]

        def wqk2(ck):
            return w_sb[:, ck, 2 * P : 3 * P]

        def wv(ck):
            return w_sb[:, ck, 3 * P : 3 * P + CP]

        wp01_sb = wl_sb[:, WP01 : WP01 + C]
        wp2_sb = wl_sb[0:HEAD, WP2 : WP2 + C]
        id_sb = wl_sb[:, IDC : IDC + P]
        mn_sb = wl_sb[:, MNC : MNC + P]
        mn2_sb = wl_sb[:, MNC : MNC + 2 * P]
        sel_sb = wl_sb[0:33, SELC : SELC + P]

        # ---- t=0 setup (no DMA dependencies; memset first so the PE warm-up
        # isn't gated behind the vector queue's DMA issues) ----
        nc.vector.memset(junk, 0.25)
        nc.vector.memset(sel32, 0.0)
        nc.vector.memset(sel32[0:1, 0:HEAD], 1.0)
        nc.vector.memset(sel32[32:33, HEAD:P], 1.0)
        nc.vector.memset(sel32b, 0.0)
        nc.vector.memset(sel32b[HEAD : HEAD + 1, :], 1.0)

        # ---- input DMAs: need-ordered, each x tchunk split across the sync
        # (HWDGE) and gpsimd (SWDGE) queues so it lands in half the time;
        # scalar carries the weights on its own HWDGE ring.
        nc.scalar.dma_start(w_sb, wqkv[:, :, :])
        for t in range(NQC):
            tsl = slice(t * QC, (t + 1) * QC)
            nc.sync.dma_start(x_sb[:, 0:3, tsl], x_t[t][:, 0:3, :])
            nc.gpsimd.dma_start(x_sb[:, 3:6, tsl], x_t[t][:, 3:6, :])
        nc.scalar.dma_start(wl_sb, wlate[:, :])
        # dummy exp triggers the ~2.7us ACT table load during the DMA fill
        nc.scalar.activation(dummy, junk[0:1, 0:8], Exp, scale=SCALE)
        # ones via ScalarE Copy(0*x+1)
        nc.scalar.activation(ones_sb, junk[0:1, 0:HEAD], Copy, bias=1.0, scale=0.0)
        # den rows live at partitions {0,32,64}; fill once so the batched
        # reciprocal never reads garbage on unused partitions.
        nc.scalar.activation(den_sb, junk[0 : HEAD + 1, :], Copy, bias=1.0, scale=0.0)
        # V ones column
        nc.scalar.activation(
            v_sb[:, :, :, HEAD : HEAD + 1],
            junk[:, 0 : TT * HPC].rearrange("p (a b) -> p a b", a=TT)[:, :, :, None],
            Copy,
            bias=1.0,
            scale=0.0,
        )
        # ---- HAM warm-up: junk matmuls spanning the DMA fill (~3..9us) so
        # the SHORT activity window fires and real work starts at 2.4GHz.
        warm = ps_sm.tile([P, QC], F32, tag="ps_sm", name="warm")
        for _ in range(N_WARM):
            nc.tensor.matmul(
                warm[:, 0:256], lhsT=junk[:, 0:P], rhs=junk[:, 0:256],
                start=True, stop=True,
            )

        BLOCKS = [(0, 1), (2,)]
        # start=True clears the WHOLE psum bank, but diagonal-shrunk score
        # matmuls only declare [c0:512) -- order them explicitly against the
        # exp that last read the recycled st slot (2 allocations ago).
        st_parity = {}
        st_count = {}
        rrs = {}
        anchors = {}  # j -> list of early att(j) TensorE instructions

        def emit_attention_blk0(j, ot_evac_sink=None, den_sink=None):
            nkt = 4 * (j + 1)
            jsl = slice(j * QC, (j + 1) * QC)
            # ---- block 0: heads 0+1, one [128,2,QC] exp per k-tile ----
            ots = [
                ps_ot.tile([HEAD + 1, QC], F32, tag=f"ot{u}", name=f"ot{u}")
                for u in range(2)
            ]
            prev = None
            for i in range(nkt + 1):
                if i < nkt:
                    m = i - 4 * j
                    c0 = m * P if m >= 0 else 0
                    st = ps_st.tile([P, 2, QC], F32, tag="st")
                    par = st_count.get("st", 0) % 2
                    st_count["st"] = st_count.get("st", 0) + 1
                    for u in range(2):
                        lo, hi = u * HEAD, (u + 1) * HEAD
                        mm = nc.tensor.matmul(
                            st[:, u, c0:QC],
                            lhsT=kt01[lo:hi, i * P : (i + 1) * P],
                            rhs=qt01[lo:hi, j * QC + c0 : (j + 1) * QC],
                            start=True,
                            stop=(m < 0),
                        )
                        if u == 0 and i == max(1, nkt // 2):
                            anchors.setdefault(j, []).append(mm.ins)
                        if c0 and st_parity.get(par) is not None:
                            add_dep_helper(mm.ins, st_parity[par], True)
                    if m >= 0:
                        # merged causal mask for both heads: one strided mm
                        nc.tensor.matmul(
                            st[:, 0:2, c0 : c0 + P],
                            lhsT=id_sb,
                            rhs=mn2_sb,
                            start=False,
                            stop=True,
                            skip_group_check=True,
                        )
                if prev is not None:
                    pi, pc0, ppt = prev
                    for u in range(2):
                        nc.tensor.matmul(
                            ots[u][:, pc0:QC],
                            lhsT=v_sb[:, pi, u, :],
                            rhs=ppt[:, u, pc0:QC],
                            start=(pi == 0),
                            stop=(pi == nkt - 1),
                        )
                if i < nkt:
                    pt = ptpool.tile([P, 2, QC], BF16, tag="pt")
                    expi = nc.scalar.activation(
                        pt[:, 0:2, c0:QC], st[:, 0:2, c0:QC], Exp, scale=SCALE
                    )
                    st_parity[par] = expi.ins
                    prev = (i, c0, pt)
            for u in range(2):

                def _dv0(u=u, ots=ots):
                    nc.vector.tensor_copy(
                        out=den_sb[32 * u : 32 * u + 1, :],
                        in_=ots[u][HEAD : HEAD + 1, :],
                    )

                if den_sink is None:
                    _dv0()
                else:
                    den_sink.append(_dv0)
                nc.vector.tensor_copy(
                    out=ot01[u * HEAD : (u + 1) * HEAD, jsl],
                    in_=ots[u][0:HEAD, :],
                )

        def emit_attention_blk1(j, ot_evac_sink=None, den_sink=None):
            nkt = 4 * (j + 1)
            jsl = slice(j * QC, (j + 1) * QC)
            # ---- block 1: head 2, exps paired across two k-tiles ----
            ot2p = ps_ot.tile([HEAD + 1, QC], F32, tag="ot0", name="ot2p")
            prevp = None
            for p in range(nkt // 2):
                i0, i1 = 2 * p, 2 * p + 1
                st = ps_st.tile([P, 2, QC], F32, tag="st")
                par = st_count.get("st", 0) % 2
                st_count["st"] = st_count.get("st", 0) + 1
                # the two score matmuls issue back-to-back on different row
                # groups (i0 from qk2 at partitions 0:64, i1 from the qk2u
                # duplicate at 64:128) so they stream concurrently
                c0s = []
                for s, i in enumerate((i0, i1)):
                    m = i - 4 * j
                    c0 = m * P if m >= 0 else 0
                    c0s.append(c0)
                    if s == 0:
                        lhsT = qk2[:, 1, i * P : (i + 1) * P]
                        rhs = qk2[:, 0, j * QC + c0 : (j + 1) * QC]
                    else:
                        lhsT = qk2u[HEAD:P, 1, i * P : (i + 1) * P]
                        rhs = qk2u[HEAD:P, 0, j * QC + c0 : (j + 1) * QC]
                    mm = nc.tensor.matmul(
                        st[:, s, c0:QC], lhsT=lhsT, rhs=rhs, start=True, stop=(m < 0)
                    )
                    if s == 0 and p == 0:
                        anchors.setdefault(j, []).append(mm.ins)
                    if c0 and st_parity.get(par) is not None:
                        add_dep_helper(mm.ins, st_parity[par], True)
                for s, i in enumerate((i0, i1)):
                    m = i - 4 * j
                    if m >= 0:
                        nc.tensor.matmul(
                            st[:, s, c0s[s] : c0s[s] + P],
                            lhsT=id_sb,
                            rhs=mn_sb,
                            start=False,
                            stop=True,
                        )
                if prevp is not None:
                    for s in range(2):
                        qi, qc0, qpt = prevp[s]
                        nc.tensor.matmul(
                            ot2p[:, qc0:QC],
                            lhsT=v_sb[:, qi, 2, :],
                            rhs=qpt[:, s, qc0:QC],
                            start=(qi == 0),
                            stop=(qi == nkt - 1),
                        )
                pt = ptpool.tile([P, 2, QC], BF16, tag="pt")
                expi = nc.scalar.activation(
                    pt[:, 0:2, c0s[0] : QC], st[:, 0:2, c0s[0] : QC], Exp, scale=SCALE
                )
                st_parity[par] = expi.ins
                prevp = ((i0, c0s[0], pt), (i1, c0s[1], pt))
            if prevp is not None:
                for s in range(2):
                    qi, qc0, qpt = prevp[s]
                    nc.tensor.matmul(
                        ot2p[:, qc0:QC],
                        lhsT=v_sb[:, qi, 2, :],
                        rhs=qpt[:, s, qc0:QC],
                        start=(qi == 0),
                        stop=(qi == nkt - 1),
                    )
            def _dv2(ot2p=ot2p):
                nc.vector.tensor_copy(
                    out=den_sb[2 * 32 : 2 * 32 + 1, :],
                    in_=ot2p[HEAD : HEAD + 1, :],
                )

            if den_sink is None:
                _dv2()
            else:
                den_sink.append(_dv2)
            nc.vector.tensor_copy(out=ot2[:, jsl], in_=ot2p[0:HEAD, :])

        def emit_recip_tail(j, h0, h1):
            # tail variant: skip the bf16 rr casts -- the bc matmuls read the
            # fp32 rec directly (fp32 lhsT consts), cutting two DVE ops and a
            # cross-engine round-trip per quarter
            hsl = slice(h0, h1)
            rec = misc.tile([HEAD + 1, QC], F32, tag="rec32", bufs=2)
            nc.vector.reciprocal(rec[:, hsl], den_sb[:, hsl])
            rrs[(j, h0)] = rec

        def emit_norm_tail(j, h0, h1):
            jsl = slice(j * QC + h0, j * QC + h1)
            hsl = slice(h0, h1)
            rec = rrs[(j, h0)]
            bc = ps_sm.tile([P, QC], F32, tag="ps_sm", name="bc")
            nc.tensor.matmul(
                bc[:, hsl], lhsT=sel32, rhs=rec[0:33, hsl], start=True, stop=True
            )
            nc.vector.tensor_mul(ot01[:, jsl], ot01[:, jsl], bc[:, hsl])
            bc2 = ps_sm.tile([P, QC], F32, tag="ps_sm", name="bc2")
            nc.tensor.matmul(
                bc2[0:HEAD, hsl],
                lhsT=sel32b,
                rhs=rec[:, hsl],
                start=True,
                stop=True,
            )
            nc.vector.tensor_mul(ot2[:, jsl], ot2[:, jsl], bc2[0:HEAD, hsl])

        def emit_recip(j, h0, h1):
            # reciprocal chain (DVE only; the custom-DVE approx ops fail this
            # walrus build's codegen, so exact InstReciprocal it is).  Emitted
            # AFTER the next chunk's QK evacuations so those don't queue
            # behind the 3.3us InstReciprocal on the DVE FIFO.  h0:h1 selects
            # a q-column half so the tail can pipeline norm/proj per half.
            hsl = slice(h0, h1)
            rec = misc.tile([HEAD + 1, QC], F32, tag="rec")
            nc.vector.reciprocal(rec[:, hsl], den_sb[:, hsl])
            rr01 = misc.tile([33, QC], BF16, tag="rr01", bufs=2)
            nc.vector.tensor_copy(out=rr01[:, hsl], in_=rec[0:33, hsl])
            rr2 = misc.tile([1, QC], BF16, tag="rr2", bufs=2)
            nc.vector.tensor_copy(out=rr2[:, hsl], in_=rec[HEAD : HEAD + 1, hsl])
            rrs[(j, h0)] = (rr01, rr2)

        def emit_norm(j, h0, h1, anchor_j=None):
            # deferred normalize: pin the bc matmuls behind early instructions
            # of att(anchor_j)'s PE stream, or the Tile scheduler slots them
            # before the reciprocal chain finishes and stalls the PE
            jsl = slice(j * QC + h0, j * QC + h1)
            hsl = slice(h0, h1)
            rr01, rr2 = rrs[(j, h0)]
            anc = anchors.get(anchor_j, []) if anchor_j is not None else []
            bc = ps_sm.tile([P, QC], F32, tag="ps_sm", name="bc")
            mm = nc.tensor.matmul(
                bc[:, hsl], lhsT=sel_sb, rhs=rr01[:, hsl], start=True, stop=True
            )
            if anc:
                add_dep_helper(mm.ins, anc[0], True)
            nc.vector.tensor_mul(ot01[:, jsl], ot01[:, jsl], bc[:, hsl])
            bc2 = ps_sm.tile([P, QC], F32, tag="ps_sm", name="bc2")
            mm = nc.tensor.matmul(
                bc2[0:HEAD, hsl], lhsT=ones_sb, rhs=rr2[:, hsl], start=True, stop=True
            )
            if anc:
                add_dep_helper(mm.ins, anc[0], True)
            nc.vector.tensor_mul(ot2[:, jsl], ot2[:, jsl], bc2[0:HEAD, hsl])

        def emit_proj(jp, i0, i1, anchor_j=None, tail=False):
            anc = anchors.get(anchor_j, []) if anchor_j is not None else []
            for i in range(i0, i1):
                isl = slice(i * P, (i + 1) * P)
                pa = ps_sm.tile([P, QC], F32, tag="ps_sm", name="pa")
                mm = nc.tensor.matmul(
                    pa, lhsT=ot01[:, isl], rhs=wp01_sb[:, 0:QC], start=True, stop=False
                )
                if anc and i == i0:
                    add_dep_helper(mm.ins, anc[-1], True)
                nc.tensor.matmul(
                    pa, lhsT=ot2[:, isl], rhs=wp2_sb[:, 0:QC], start=False, stop=True
                )
                y_sb = ypool.tile([P, C], BF16, tag="ysb")
                pb = ps_sm.tile([P, QC], F32, tag="ps_sm", name="pb")
                nc.tensor.matmul(
                    pb[:, : C - QC],
                    lhsT=ot01[:, isl],
                    rhs=wp01_sb[:, QC:C],
                    start=True,
                    stop=False,
                )
                nc.tensor.matmul(
                    pb[:, : C - QC],
                    lhsT=ot2[:, isl],
                    rhs=wp2_sb[:, QC:C],
                    start=False,
                    stop=True,
                )
                if tail:
                    # ScalarE is idle once the last exp retires: alternate
                    # the tail evacuations between ScalarE and DVE so they
                    # run 2-wide, and drain on both HWDGE rings (sync=SP,
                    # scalar=ACT).  The final tile's DMA is split across
                    # both rings so the last transfer runs 2-wide.
                    if i % 2 == 0:
                        nc.scalar.copy(out=y_sb[:, 0:QC], in_=pa)
                        nc.scalar.copy(out=y_sb[:, QC:C], in_=pb[:, : C - QC])
                    else:
                        nc.vector.tensor_copy(out=y_sb[:, 0:QC], in_=pa)
                        nc.vector.tensor_copy(out=y_sb[:, QC:C], in_=pb[:, : C - QC])
                    if i == T // P - 1:
                        nc.sync.dma_start(y[isl, 0:QC], y_sb[:, 0:QC])
                        nc.scalar.dma_start(y[isl, QC:C], y_sb[:, QC:C])
                        continue
                    eng = nc.sync if i % 2 == 0 else nc.scalar
                else:
                    nc.vector.tensor_copy(out=y_sb[:, 0:QC], in_=pa)
                    nc.vector.tensor_copy(out=y_sb[:, QC:C], in_=pb[:, : C - QC])
                    eng = nc.sync
                eng.dma_start(y[isl, :], y_sb)

        def emit_vqk(j, first_dep=None):
            jsl = slice(j * QC, (j + 1) * QC)
            # ---- V t-tiles for this q-chunk ----
            for i in range(4 * j, 4 * j + 4):
                pv = ps_sm.tile([P, QC], F32, tag="ps_sm", name="pv")
                for ci in range(CK):
                    mm = nc.tensor.matmul(
                        pv[:, :CP],
                        lhsT=x_sb[:, ci, i * P : (i + 1) * P],
                        rhs=wv(ci),
                        start=(ci == 0),
                        stop=(ci == CK - 1),
                    )
                    if first_dep is not None:
                        add_dep_helper(mm.ins, first_dep, True)
                        first_dep = None
                nc.vector.tensor_copy(
                    out=v_sb[:, i, :, 0:HEAD],
                    in_=pv[:, :CP].rearrange("p (h d) -> p h d", d=HEAD),
                )
            # ---- QK chunk j.  q-chains first: att(j)'s first scores need
            # qt chunk j but only OLD kt chunks (k-tile 4j comes last) ----
            for wf, dsts in (
                (wq, ((qt01[:, jsl], slice(0, P)),)),
                (wqk2, ((qk2[:, 0, jsl], slice(0, HEAD)), (qk2[:, 1, jsl], slice(HEAD, P)))),
                (wk, ((kt01[:, jsl], slice(0, P)),)),
            ):
                pq = ps_sm.tile([P, QC], F32, tag="ps_sm", name="pq")
                for ci in range(CK):
                    nc.tensor.matmul(
                        pq,
                        lhsT=wf(ci),
                        rhs=x_sb[:, ci, jsl],
                        start=(ci == 0),
                        stop=(ci == CK - 1),
                    )
                for dst, psl in dsts:
                    nc.vector.tensor_copy(out=dst, in_=pq[psl, :])
            nc.sync.dma_start(qk2u[HEAD:P, :, jsl], qk2[:, :, jsl])

        emit_vqk(0)
        tail_sink = []
        den_sink = []
        for j in range(NQC):
            # attention(j), with the NEXT chunk's V/QK emitted between the
            # two head blocks: their PE chains fill blk1's exp gaps and
            # their DVE evacuations retire during blk1, so att(j+1)'s first
            # scores never stall on the DVE FIFO at the chunk boundary.
            # The reciprocal goes after all of it for the same reason.
            last = j == NQC - 1
            b0 = emit_attention_blk0(
                j,
                ot_evac_sink=tail_sink if last else None,
                den_sink=den_sink if last else None,
            )
            if j == 0:
                # att(0) has no previous chunk to project, so the x_t1 DMA
                # wait ahead of vqk(1) is a genuine PE hole: bridge it with
                # junk matmuls so the clock gate doesn't re-throttle (a cold
                # window right after this stall costs 3-10us at half clock)
                br0 = ps_sm.tile([P, QC], F32, tag="ps_sm", name="br0")
                br0_last = None
                for k in range(32):
                    mm = nc.tensor.matmul(
                        br0[:, 0:256], lhsT=junk[:, 0:P], rhs=junk[:, 0:256],
                        start=True, stop=True,
                    )
                    if k == 0:
                        add_dep_helper(mm.ins, b0, True)
                    br0_last = mm.ins
            if j + 1 < NQC:
                # pin vqk(1)'s first matmul behind the bridge so the FIFO
                # drains the junk DURING the x_t1 wait, not after it
                emit_vqk(j + 1, first_dep=br0_last if j == 0 else None)
            b1 = emit_attention_blk1(
                j,
                ot_evac_sink=tail_sink if last else None,
                den_sink=den_sink if last else None,
            )
            if j < NQC - 1:
                emit_recip(j, 0, QC)
                if j >= 1:
                    emit_norm(j - 1, 0, QC, anchor_j=j)
                    emit_proj(j - 1, 4 * (j - 1), 4 * j, anchor_j=j)
            else:
                # last chunk: norm/proj of j-1 first (they fill att(j)'s exp
                # gaps and their DVE work must sit AHEAD of the tail's den
                # copies in the FIFO), then flush the deferred den copies
                emit_norm(j - 1, 0, QC, anchor_j=j)
                emit_proj(j - 1, 4 * (j - 1), 4 * j, anchor_j=j)
                for fn in den_sink:
                    fn()
        # tail: per-i-tile pipeline.  The denominator rows are already on
        # the DVE queue; the first quarter's reciprocal runs BEFORE the ot
        # evacuations (deferred into tail_sink) so norm/proj/drain of tile
        # 4*jL starts as early as possible, and each subsequent quarter's
        # reciprocal overlaps the previous quarter's projection and drain.
        jL = NQC - 1
        Q4 = P
        # single surgical HAM bridge: keep the PE busy across the ~2.5us
        # reciprocal/den wait after att(3)'s last PV so the tail projections
        # run at 2.4GHz instead of re-throttled half clock.  Chained WAW on
        # one scratch tile and pinned behind the last real PE instruction.
        brt = ps_sm.tile([P, QC], F32, tag="ps_sm", name="brtail")
        prev_mm = None
        for k in range(24):
            mm = nc.tensor.matmul(
                brt[:, 0:256], lhsT=junk[:, 0:P], rhs=junk[:, 0:256],
                start=True, stop=True,
            )
            if k == 0:
                add_dep_helper(mm.ins, b1, True)
            prev_mm = mm
        emit_recip_tail(jL, 0, Q4)
        for fn in tail_sink:
            fn()
        for q in range(4):
            if q:
                emit_recip_tail(jL, q * Q4, (q + 1) * Q4)
            emit_norm_tail(jL, q * Q4, (q + 1) * Q4)
            emit_proj(jL, 4 * jL + q, 4 * jL + q + 1, tail=True)

    _split_excess_waits(nc)
    return nc


def _in_maps(x, W_attn, W_proj):
    import ml_dtypes

    bf = ml_dtypes.bfloat16
    ident = np.eye(P, dtype=np.float32)
    mneg = np.where(
        np.arange(P)[:, None] > np.arange(P)[None, :], NEG, 0.0
    ).astype(np.float32)
    sel = np.zeros((33, P), dtype=np.float32)
    sel[0, 0:HEAD] = 1.0
    sel[32, HEAD:P] = 1.0
    Wq, Wk, Wv = W_attn[0:C], W_attn[C : 2 * C], W_attn[2 * C : 3 * C]
    maps = []
    for core in range(8):
        b, g = divmod(core, 4)
        s = slice(g * CP, (g + 1) * CP)
        wq = Wq[s].T  # [C, 192]
        wk = Wk[s].T
        wv = Wv[s].T
        wp = W_proj[:, s].T  # [192, C]
        # wqkv pack: [128, CK, 576] = per ck: [wq01 | wk01 | wqk2 | wv]
        wcat = np.concatenate(
            [
                wq[:, 0:P],
                wk[:, 0:P],
                np.concatenate([wq[:, P:CP], wk[:, P:CP]], axis=1),
                wv,
            ],
            axis=1,
        )  # [C, 576]
        wqkv = np.ascontiguousarray(
            wcat.reshape(CK, P, 576).transpose(1, 0, 2)
        ).astype(bf)
        # wlate pack: [128, WLW]
        wl = np.zeros((P, WLW), dtype=np.float32)
        wl[:, WP01 : WP01 + C] = wp[0:P]
        wl[0:HEAD, WP2 : WP2 + C] = wp[P:CP]
        wl[:, IDC : IDC + P] = ident
        wl[:, MNC : MNC + P] = mneg
        wl[:, MNC + P : MNC + 2 * P] = mneg
        wl[0:33, SELC : SELC + P] = sel
        # x tchunk packs: [128, CK, QC]
        xb = np.ascontiguousarray(x[b].T).astype(bf).reshape(CK, P, T)
        m = dict(
            wqkv=wqkv,
            wlate=wl.astype(bf),
        )
        for t in range(NQC):
            m[f"x_t{t}"] = np.ascontiguousarray(
                xb[:, :, t * QC : (t + 1) * QC].transpose(1, 0, 2)
            )
        maps.append(m)
    return maps


def run(x, W_attn, W_proj, trace=False):
    if "nc" not in _CACHED:
        _CACHED["nc"] = _build()
    nc = _CACHED["nc"]
    res = run_bass_kernel_spmd(nc, _in_maps(x, W_attn, W_proj), list(range(8)), trace=trace)
    y = np.empty((B, T, C), dtype=np.float32)
    for b in range(B):
        y[b] = np.asarray(res.results[4 * b]["y"], dtype=np.float32)
        for g in range(1, 4):
            y[b] += np.asarray(res.results[4 * b + g]["y"], dtype=np.float32)
    return y, res


def kernel(x, W_attn, W_proj):
    x = np.asarray(x, dtype=np.float32)
    W_attn = np.asarray(W_attn, dtype=np.float32)
    W_proj = np.asarray(W_proj, dtype=np.float32)
    y, _ = run(x, W_attn, W_proj, trace=False)
    return y
